# revision 2
# baseline (speedup 1.0000x reference)
"""Bass/Trainium2 kernel for a 2-layer multi-head GAT (DocRE model).

Contract: kernel(**inputs) takes the FULL unsharded inputs as numpy arrays
and returns the FULL [512, 768] float32 output. Internally the 512 nodes are
row-sharded across 8 NeuronCores; per-head weights are replicated; the small
x1 activation is AllGathered on-device between the two layers.

Key structure (v2):
- e is pre-transposed on the host to [i, k, j] and cast to bf16 so the big
  edge-score contraction streams at full HBM bandwidth with features on the
  partition axis. Both layers' edge scores come from ONE pass over e using
  the host-folded [768, 24] matrix V = [We0(W0 a0_3) | We1(W1 a1_3)].
- Layer-1 output is restructured to avoid the big h1 AllGather:
      out_i = elu( (1/H) sum_h (att1[h] @ x1) @ W1[h] )
  Each core computes y[h] = att1[h, local rows] @ x1 (needs only the small
  gathered x1), then contracts with replicated streamed W1. No 9.4MB
  collective.
- Layer-0 additive score terms (s_src + s_dst + adj mask) are host-folded
  into ha0; layer-1's mask is host-folded into mk1.
"""

import sys

sys.path.insert(0, "/opt/trn_rl_repo")

import numpy as np
import ml_dtypes

from concourse import bass, bacc, mybir, tile
from concourse.bass_utils import run_bass_kernel_spmd

BF16 = ml_dtypes.bfloat16

N = 512          # nodes
D = 768          # hidden
H = 12           # heads
F0 = 64          # layer-0 per-head dim
NCORES = 8
NPC = N // NCORES          # 64 local rows per core
NBLK = NPC // 4            # 16 blocks of 4 rows
ALPHA = 0.2
KT = D // 128              # 6 contraction tiles

F32 = mybir.dt.float32
BF = mybir.dt.bfloat16
ADD = mybir.AluOpType.add
MULT = mybir.AluOpType.mult
AF = mybir.ActivationFunctionType

_COMPILED = None
DEBUG = False
SIM_SAFE = False  # replace Prelu (not in interp) with Relu for cost-model sims


def _build_nc():
    nc = bacc.Bacc("TRN2", target_bir_lowering=False, num_devices=NCORES)
    lrelu_fn = AF.Relu if SIM_SAFE else AF.Prelu

    eT_d = nc.dram_tensor("eT", [NPC, D, N], BF, kind="ExternalInput")
    xT_d = nc.dram_tensor("xT", [128, KT * N], BF, kind="ExternalInput")
    w0r_d = nc.dram_tensor("w0r", [128, KT * D], BF, kind="ExternalInput")
    w1t_d = nc.dram_tensor("w1t", [H, 128, KT * D], BF, kind="ExternalInput")
    v_d = nc.dram_tensor("vw", [128, KT * 32], BF, kind="ExternalInput")
    u1_d = nc.dram_tensor("u1", [128, KT * 24], BF, kind="ExternalInput")
    ha0_d = nc.dram_tensor("ha0", [NBLK, 128, N], BF, kind="ExternalInput")
    mk1_d = nc.dram_tensor("mk1", [NBLK, 128, N], BF, kind="ExternalInput")
    ident_d = nc.dram_tensor("ident", [128, 128], BF, kind="ExternalInput")

    out_d = nc.dram_tensor("out", [NPC, D], F32, kind="ExternalOutput")

    agx_in = nc.dram_tensor("agx_in", [NPC, D], BF)
    agx_out = nc.dram_tensor("agx_out", [N, D], BF, addr_space="Shared")

    with tile.TileContext(nc) as tc:
        with (
            tc.tile_pool(name="const", bufs=1) as constp,
            tc.tile_pool(name="pers", bufs=1) as pers,
            tc.tile_pool(name="hapool", bufs=3) as hapool,
            tc.tile_pool(name="w1pool", bufs=3) as w1pool,
            tc.tile_pool(name="mkpool", bufs=3) as mkpool,
        ):
            w1tiles = []
            ident = constp.tile([128, 128], BF, tag="ident")
            nc.scalar.dma_start(out=ident[:, :], in_=ident_d[:, :])
            u1 = constp.tile([128, KT * 24], BF, tag="u1")
            nc.scalar.dma_start(out=u1[:, :], in_=u1_d[:, :])

            se1st = [pers.tile([128, N], BF, tag=f"se1_{b}", name=f"se1_{b}") for b in range(NBLK)]
            x1T = pers.tile([128, KT * N], BF, tag="x1T")
            s1loc = pers.tile([24, NPC], F32, tag="s1loc")
            dsta1 = pers.tile([128, N], F32, tag="dsta1")
            src1c = pers.tile([128, NBLK], F32, tag="src1c")
            x1f = [pers.tile([128, D], BF, tag=f"x1f_{m}", name=f"x1f_{m}") for m in range(4)]

            # =================== layer 0 ===================
            with (
                tc.tile_pool(name="l0pers", bufs=1) as l0p,
                tc.tile_pool(name="l0const", bufs=1) as l0c,
                tc.tile_pool(name="epool", bufs=3) as epool,
                tc.tile_pool(name="l0work", bufs=2) as work,
            ):
                xT = l0c.tile([128, KT * N], BF, tag="xT")
                nc.gpsimd.dma_start(out=xT[:, :], in_=xT_d[:, :])
                w0r = l0c.tile([128, KT * D], BF, tag="w0r")
                nc.gpsimd.dma_start(out=w0r[:, :], in_=w0r_d[:, :])
                vw = l0c.tile([128, KT * 32], BF, tag="vw")
                nc.sync.dma_start(out=vw[:, :], in_=v_d[:, :])

                # ---- e-pass: scores + softmax + att0^T, 16 blocks ----
                # at0T columns: [q-chunk (2048): 128b + 32c + r]
                h0 = [l0p.tile([128, D], BF, tag=f"h0_{m}", name=f"h0_{m}") for m in range(4)]
                at0T = l0p.tile([128, 4 * NBLK * 128], BF, tag="at0T")
                with (
                    tc.tile_pool(name="psh0", bufs=1, space="PSUM") as psh0,
                    tc.tile_pool(name="psx1", bufs=1, space="PSUM") as psx1,
                    tc.tile_pool(name="psv", bufs=2, space="PSUM") as psvp,
                    tc.tile_pool(name="pst", bufs=2, space="PSUM") as pstp,
                ):
                    def emit_h0():
                        # h0 = x @ W0r -> [4][128 nodes, 768] bf16
                        for m in range(4):
                            pa = psh0.tile([128, 512], F32, tag="ph0a")
                            pb = psh0.tile([128, 256], F32, tag="ph0b")
                            for k in range(KT):
                                lhs = xT[:, k * N + 128 * m : k * N + 128 * (m + 1)]
                                nc.tensor.matmul(
                                    pa[:, :], lhs, w0r[:, k * D : k * D + 512],
                                    start=(k == 0), stop=(k == KT - 1),
                                )
                                nc.tensor.matmul(
                                    pb[:, :], lhs, w0r[:, k * D + 512 : (k + 1) * D],
                                    start=(k == 0), stop=(k == KT - 1),
                                )
                            nc.vector.tensor_copy(out=h0[m][:, 0:512], in_=pa[:, :])
                            nc.vector.tensor_copy(out=h0[m][:, 512:768], in_=pb[:, :])

                    def emit_block(b):
                        eng = nc.sync if b % 2 == 0 else nc.gpsimd
                        mkeng = nc.gpsimd if b % 2 == 0 else nc.sync
                        et = epool.tile([128, KT * 4 * N], BF, tag="etile")
                        eng.dma_start(
                            out=et[:, :].rearrange(
                                "p (c kb j) -> p c kb j", kb=KT, c=4
                            ),
                            in_=eT_d[4 * b : 4 * b + 4].rearrange(
                                "c (kb p) j -> p c kb j", p=128
                            ),
                        )
                        ha = hapool.tile([128, N], BF, tag="ha0")
                        nc.scalar.dma_start(out=ha[:, :], in_=ha0_d[b])
                        mk = mkpool.tile([128, N], BF, tag="mk1")
                        mkeng.dma_start(out=mk[:, :], in_=mk1_d[b])

                        psv = psvp.tile([128, N], F32, tag="psv")
                        for cc in range(4):
                            for k in range(KT):
                                nc.tensor.matmul(
                                    psv[32 * cc : 32 * cc + 32, :],
                                    vw[:, 32 * k : 32 * (k + 1)],
                                    et[:, (cc * KT + k) * N : (cc * KT + k + 1) * N],
                                    start=(k == 0), stop=(k == KT - 1),
                                    tile_position=(0, 32 * cc),
                                )
                        # layer-1 score base: e-term + adj mask (folded here so
                        # the layer-1 block loop saves one tensor_tensor)
                        nc.vector.tensor_tensor(
                            out=se1st[b][:, :], in0=psv[:, :], in1=mk[:, :],
                            op=ADD,
                        )
                        sc0 = work.tile([128, N], F32, tag="sc0")
                        nc.vector.tensor_tensor(
                            out=sc0[:, :], in0=psv[:, :], in1=ha[:, :], op=ADD
                        )
                        lr0 = work.tile([128, N], F32, tag="lr0")
                        nc.scalar.activation(
                            lr0[:, :], sc0[:, :], lrelu_fn, alpha=ALPHA
                        )
                        ex0 = work.tile([128, N], F32, tag="ex0")
                        z0 = work.tile([128, 1], F32, tag="z0")
                        nc.scalar.activation(
                            ex0[:, :], lr0[:, :], AF.Exp, accum_out=z0[:, :]
                        )
                        rz0 = work.tile([128, 1], F32, tag="rz0")
                        nc.vector.reciprocal(rz0[:, :], z0[:, :])
                        at0 = work.tile([128, N], BF, tag="at0")
                        nc.vector.tensor_scalar(
                            out=at0[:, :], in0=ex0[:, :], scalar1=rz0[:, :],
                            scalar2=None, op0=MULT,
                        )
                        pt = pstp.tile([128, 512], BF, tag="ptr")
                        for q in range(4):
                            nc.tensor.transpose(
                                pt[:, 128 * q : 128 * (q + 1)],
                                at0[:, 128 * q : 128 * (q + 1)], ident[:, :],
                            )
                        nc.vector.tensor_copy(
                            out=at0T[:, :].rearrange(
                                "p (q col) -> p q col", q=4
                            )[:, :, 128 * b : 128 * (b + 1)],
                            in_=pt[:, :].rearrange("p (q col) -> p q col", q=4),
                        )

                    x1p = work.tile([64, D], F32, tag="x1p", bufs=1)
                    tmin = work.tile([64, D], F32, tag="tmin", bufs=1)
                    texp = work.tile([64, D], F32, tag="texp", bufs=1)
                    tmax = work.tile([64, D], F32, tag="tmax", bufs=1)
                    x1bf = work.tile([64, D], BF, tag="x1bf", bufs=1)

                    def emit_x1_chunk(ck):
                        # x1 rows 32ck..32ck+32 = elu(att0 @ h0) for 8 blocks
                        r0 = 32 * ck
                        pa = psx1.tile([32, 512], F32, tag="px1a")
                        pb = psx1.tile([32, 256], F32, tag="px1b")
                        for h in range(H):
                            dst = (
                                pa[:, 64 * h : 64 * (h + 1)]
                                if h < 8
                                else pb[:, 64 * (h - 8) : 64 * (h - 7)]
                            )
                            for q in range(4):
                                lhs = at0T[:, :].rearrange(
                                    "p (q b c r) -> p q b c r", q=4, b=NBLK, c=4
                                )[:, q, 8 * ck : 8 * ck + 8, :, h : h + 1]
                                nc.tensor.matmul(
                                    dst, lhs, h0[q][:, 64 * h : 64 * (h + 1)],
                                    start=(q == 0), stop=(q == 3),
                                )
                        sl = slice(r0, r0 + 32)
                        nc.vector.tensor_copy(out=x1p[sl, 0:512], in_=pa[:, :])
                        nc.vector.tensor_copy(out=x1p[sl, 512:768], in_=pb[:, :])
                        nc.vector.tensor_scalar(
                            out=tmin[sl, :], in0=x1p[sl, :], scalar1=0.0,
                            scalar2=None, op0=mybir.AluOpType.min,
                        )
                        nc.scalar.activation(texp[sl, :], tmin[sl, :], AF.Exp)
                        nc.vector.tensor_scalar(
                            out=tmax[sl, :], in0=x1p[sl, :], scalar1=0.0,
                            scalar2=None, op0=mybir.AluOpType.max,
                        )
                        nc.vector.scalar_tensor_tensor(
                            out=x1bf[sl, :], in0=texp[sl, :], scalar=-1.0,
                            in1=tmax[sl, :], op0=ADD, op1=ADD,
                        )
                        nc.scalar.dma_start(
                            out=agx_in[sl, :], in_=x1bf[sl, :]
                        )

                    emit_block(0)
                    emit_h0()
                    for b in range(1, NBLK):
                        emit_block(b)
                        if b % 8 == 7:
                            emit_x1_chunk(b // 8)

                # ---- AllGather x1 (issued first; local work overlaps it) ----
                nc.gpsimd.collective_compute(
                    "AllGather", mybir.AluOpType.bypass,
                    replica_groups=[list(range(NCORES))],
                    ins=[agx_in.ap().opt()], outs=[agx_out.ap().opt()],
                )
                # prefetch W1 head tiles on the Act queue (SP/Pool carry et)
                for h in range(H):
                    w1t = w1pool.tile([128, KT * D], BF, tag="w1t", name=f"w1t_{h}")
                    weng = nc.scalar if h < 3 else (nc.sync if h % 2 else nc.gpsimd)
                    weng.dma_start(out=w1t[:, :], in_=w1t_d[h])
                    w1tiles.append(w1t)

                # local x1^T for s_src1 (overlaps the collective)
                x1locT = l0p.tile([128, KT * NPC], BF, tag="x1locT")
                with tc.tile_pool(name="pslt", bufs=2, space="PSUM") as pslt:
                    for k6 in range(KT):
                        pt = pslt.tile([128, 64], BF, tag="plt")
                        nc.tensor.transpose(
                            pt[:, 0:64],
                            x1bf[:, 128 * k6 : 128 * (k6 + 1)],
                            ident[0:64, 0:64],
                        )
                        nc.vector.tensor_copy(
                            out=x1locT[:, NPC * k6 : NPC * (k6 + 1)], in_=pt[:, 0:64]
                        )
                    psl = pslt.tile([24, NPC], F32, tag="psl")
                    for k in range(KT):
                        nc.tensor.matmul(
                            psl[:, :], u1[:, 24 * k : 24 * (k + 1)],
                            x1locT[:, NPC * k : NPC * (k + 1)],
                            start=(k == 0), stop=(k == KT - 1),
                        )
                    nc.vector.tensor_copy(out=s1loc[:, :], in_=psl[:, :])
                nc.vector.memset(dsta1[:, :], 0.0)
                nc.vector.memset(src1c[:, :], 0.0)
                for cc in range(4):
                    nc.sync.dma_start(
                        out=src1c[32 * cc + 12 : 32 * cc + 24, :],
                        in_=s1loc[0:12, :].rearrange(
                            "h (b c) -> h b c", c=4
                        )[:, :, cc : cc + 1],
                    )

                # gathered x1 arrives: x1f node-major copies + x1^T + s_dst1
                for m in range(4):
                    feng = nc.sync if m % 2 == 0 else nc.gpsimd
                    feng.dma_start(
                        out=x1f[m][:, :], in_=agx_out[128 * m : 128 * (m + 1), :]
                    )
                with tc.tile_pool(name="psxt", bufs=2, space="PSUM") as psxt:
                    for m in range(4):
                        pt = psxt.tile([128, KT * 128], BF, tag="pxt")
                        for k6 in range(KT):
                            nc.tensor.transpose(
                                pt[:, 128 * k6 : 128 * (k6 + 1)],
                                x1f[m][:, 128 * k6 : 128 * (k6 + 1)],
                                ident[:, :],
                            )
                        nc.vector.tensor_copy(
                            out=x1T[:, :].rearrange(
                                "p (k col) -> p k col", k=KT
                            )[:, :, 128 * m : 128 * (m + 1)],
                            in_=pt[:, :].rearrange("p (k col) -> p k col", k=KT),
                        )

                # ---- s_dst1 for all nodes ----
                with tc.tile_pool(name="pss1", bufs=1, space="PSUM") as pss1:
                    ps1 = pss1.tile([24, N], F32, tag="ps1")
                    for k in range(KT):
                        nc.tensor.matmul(
                            ps1[:, :], u1[:, 24 * k : 24 * (k + 1)],
                            x1T[:, N * k : N * (k + 1)],
                            start=(k == 0), stop=(k == KT - 1),
                        )
                    s1 = work.tile([24, N], F32, tag="s1", bufs=1)
                    nc.vector.tensor_copy(out=s1[:, :], in_=ps1[:, :])
                    for cc in range(4):
                        deng = nc.sync if cc % 2 == 0 else nc.gpsimd
                        deng.dma_start(
                            out=dsta1[32 * cc + 12 : 32 * cc + 24, :],
                            in_=s1[12:24, :],
                        )

            # =================== layer 1 ===================
            with (
                tc.tile_pool(name="l1pers", bufs=1) as l1p,
                tc.tile_pool(name="l1work", bufs=4) as work,
            ):
                at1T = l1p.tile([128, 4 * NBLK * 128], BF, tag="at1T")
                with tc.tile_pool(name="pst1", bufs=2, space="PSUM") as pstp:
                    for b in range(NBLK):
                        t1 = work.tile([128, N], F32, tag="t1")
                        nc.vector.scalar_tensor_tensor(
                            out=t1[:, :], in0=se1st[b][:, :],
                            scalar=src1c[:, b : b + 1], in1=dsta1[:, :],
                            op0=ADD, op1=ADD,
                        )
                        lr1 = work.tile([128, N], F32, tag="lr1")
                        nc.scalar.activation(
                            lr1[:, :], t1[:, :], lrelu_fn, alpha=ALPHA
                        )
                        ex1 = work.tile([128, N], F32, tag="ex1")
                        z1 = work.tile([128, 1], F32, tag="z1")
                        nc.scalar.activation(
                            ex1[:, :], lr1[:, :], AF.Exp, accum_out=z1[:, :]
                        )
                        rz1 = work.tile([128, 1], F32, tag="rz1")
                        nc.vector.reciprocal(rz1[:, :], z1[:, :])
                        at1 = work.tile([128, N], BF, tag="at1")
                        nc.vector.tensor_scalar(
                            out=at1[:, :], in0=ex1[:, :], scalar1=rz1[:, :],
                            scalar2=None, op0=MULT,
                        )
                        pt = pstp.tile([128, 512], BF, tag="ptr1")
                        for q in range(4):
                            nc.tensor.transpose(
                                pt[:, 128 * q : 128 * (q + 1)],
                                at1[:, 128 * q : 128 * (q + 1)], ident[:, :],
                            )
                        nc.vector.tensor_copy(
                            out=at1T[:, :].rearrange(
                                "p (q col) -> p q col", q=4
                            )[:, :, 128 * b : 128 * (b + 1)],
                            in_=pt[:, :].rearrange("p (q col) -> p q col", q=4),
                        )

                # ---- stage 1: yT[k] = x1f^T-chunks contracted with at1^T ----
                # yT columns ordered (b, c, h): col = 48b + 12c + h
                yT = l1p.tile([128, KT * D], BF, tag="yT")
                with tc.tile_pool(name="psy", bufs=2, space="PSUM") as psy:
                    for k in range(KT):
                        py = [
                            psy.tile([128, 384], F32, tag="pya", name="pya"),
                            psy.tile([128, 384], F32, tag="pyb", name="pyb"),
                        ]
                        for q in range(4):
                            for half in range(2):
                                rhs = at1T[:, :].rearrange(
                                    "p (q b c r) -> p q b c r", q=4, b=NBLK, c=4
                                )[:, q, 8 * half : 8 * (half + 1), :, 12:24]
                                nc.tensor.matmul(
                                    py[half][:, :],
                                    x1f[q][:, 128 * k : 128 * (k + 1)],
                                    rhs,
                                    start=(q == 0), stop=(q == 3),
                                )
                        for half in range(2):
                            nc.vector.tensor_copy(
                                out=yT[:, D * k + 384 * half : D * k + 384 * (half + 1)],
                                in_=py[half][:, :],
                            )

                # ---- stage 2: out = elu(sum_h y[h] @ W1[h] / H) ----
                with tc.tile_pool(name="pso", bufs=1, space="PSUM") as psop:
                    po = [
                        psop.tile([64, 384], F32, tag="po0", name="po0"),
                        psop.tile([64, 384], F32, tag="po1", name="po1"),
                    ]
                    for h in range(H):
                        w1t = w1tiles[h]
                        for k in range(KT):
                            lhs = yT[:, :].rearrange(
                                "p (k b c r) -> p k b c r", k=KT, b=NBLK, c=4
                            )[:, k, :, :, h : h + 1]
                            for half in range(2):
                                nc.tensor.matmul(
                                    po[half][:, :], lhs,
                                    w1t[:, D * k + 384 * half : D * k + 384 * (half + 1)],
                                    start=(h == 0 and k == 0),
                                    stop=(h == H - 1 and k == KT - 1),
                                )
                    op = work.tile([64, D], F32, tag="op")
                    for half in range(2):
                        nc.vector.tensor_copy(
                            out=op[:, 384 * half : 384 * (half + 1)], in_=po[half][:, :]
                        )
                omin = work.tile([64, D], F32, tag="omin")
                nc.vector.tensor_scalar(
                    out=omin[:, :], in0=op[:, :], scalar1=0.0, scalar2=None,
                    op0=mybir.AluOpType.min,
                )
                oexp = work.tile([64, D], F32, tag="oexp")
                nc.scalar.activation(oexp[:, :], omin[:, :], AF.Exp)
                omax = work.tile([64, D], F32, tag="omax")
                nc.vector.tensor_scalar(
                    out=omax[:, :], in0=op[:, :], scalar1=0.0, scalar2=None,
                    op0=mybir.AluOpType.max,
                )
                ofin = work.tile([64, D], F32, tag="ofin")
                nc.vector.scalar_tensor_tensor(
                    out=ofin[:, :], in0=oexp[:, :], scalar=-1.0, in1=omax[:, :],
                    op0=ADD, op1=ADD,
                )
                nc.scalar.dma_start(out=out_d[:, :], in_=ofin[:, :])

    nc.compile()
    return nc


def _fold_weights(We, W, a, F_):
    We = We.astype(np.float64)
    W = W.astype(np.float64)
    a = a.astype(np.float64)
    a1, a2, a3 = a[:, :F_], a[:, F_ : 2 * F_], a[:, 2 * F_ :]
    v = np.einsum("hei,hif,hf->he", We, W, a3)
    usrc = np.einsum("hif,hf->hi", W, a1)
    udst = np.einsum("hif,hf->hi", W, a2)
    return v, usrc, udst


def _to_ktile(mat):
    """[768, C] -> [128, KT*C] with the KT k-tiles side by side."""
    k, c = mat.shape
    assert k == D
    return np.ascontiguousarray(
        mat.reshape(KT, 128, c).transpose(1, 0, 2).reshape(128, KT * c)
    )


def kernel(**inputs):
    global _COMPILED
    x = np.asarray(inputs["x"], dtype=np.float32)
    adj = np.asarray(inputs["adj"])
    e = np.asarray(inputs["e"], dtype=np.float32)
    W0 = np.asarray(inputs["W0"], dtype=np.float32)
    a0 = np.asarray(inputs["a0"], dtype=np.float32)
    W1 = np.asarray(inputs["W1"], dtype=np.float32)
    a1_ = np.asarray(inputs["a1"], dtype=np.float32)
    We0 = np.asarray(inputs["We0"], dtype=np.float32)
    We1 = np.asarray(inputs["We1"], dtype=np.float32)

    v0, _, _ = _fold_weights(We0, W0, a0, F0)
    v1, u1src, u1dst = _fold_weights(We1, W1, a1_, D)
    V = np.concatenate([v0, v1], axis=0).T.astype(np.float32)        # [768, 24]
    U1 = np.concatenate([u1src, u1dst], axis=0).T.astype(np.float32)  # [768, 24]

    h0h = np.einsum("ni,hif->hnf", x.astype(np.float64), W0.astype(np.float64))
    s_src0 = np.einsum("hnf,hf->hn", h0h, a0[:, :F0].astype(np.float64))
    s_dst0 = np.einsum("hnf,hf->hn", h0h, a0[:, F0 : 2 * F0].astype(np.float64))
    maskadd = (adj.astype(np.float32) - 1.0) * 9e15                   # 0 or -9e15

    xT_bf = _to_ktile(np.ascontiguousarray(x.T)).astype(BF16)
    w0r_bf = _to_ktile(W0.transpose(1, 0, 2).reshape(D, H * F0)).astype(BF16)
    Vp = np.zeros((D, 32), np.float32)
    Vp[:, :24] = V
    v_bf = _to_ktile(Vp).astype(BF16)
    u1_bf = _to_ktile(U1).astype(BF16)
    ident = np.eye(128, dtype=np.float32).astype(BF16)
    eT = np.ascontiguousarray(e.transpose(0, 2, 1)).astype(BF16)      # [N, D, N]

    # per-head k-tiled W1 with the 1/H mean folded in: [H, 128, KT*D]
    w1t_bf = np.stack(
        [_to_ktile(W1[h] / H).astype(BF16) for h in range(H)], axis=0
    )

    in_maps = []
    for c in range(NCORES):
        ha0 = np.zeros((NBLK, 128, N), dtype=np.float32)
        mk1 = np.zeros((NBLK, 128, N), dtype=np.float32)
        for b in range(NBLK):
            for cc in range(4):
                i = NPC * c + 4 * b + cc
                ha0[b, 32 * cc : 32 * cc + 12, :] = (
                    s_dst0 + s_src0[:, i : i + 1] + maskadd[i : i + 1, :]
                )
                mk1[b, 32 * cc + 12 : 32 * cc + 24, :] = maskadd[i : i + 1, :]
        in_maps.append(
            {
                "eT": eT[NPC * c : NPC * (c + 1)],
                "xT": xT_bf,
                "w0r": w0r_bf,
                "w1t": w1t_bf,
                "vw": v_bf,
                "u1": u1_bf,
                "ha0": ha0.astype(BF16),
                "mk1": mk1.astype(BF16),
                "ident": ident,
            }
        )

    if _COMPILED is None:
        _COMPILED = _build_nc()
    nc = _COMPILED

    res = run_bass_kernel_spmd(nc, in_maps, list(range(NCORES)))
    out = np.concatenate([res.results[c]["out"] for c in range(NCORES)], axis=0)
    return out.astype(np.float32)


if __name__ == "__main__":
    import reference

    inputs = {k: np.asarray(v) for k, v in reference.setup_inputs().items()}
    got = kernel(**inputs)
    print("output shape:", got.shape, got.dtype)
